# revision 1
# baseline (speedup 1.0000x reference)
"""EnsembleMLP fused kernel for Trainium2, 8 NeuronCores (SPMD, batch-parallel).

Math transformation
-------------------
reference:
    hidden = tanh(x @ W_in.T)                                   [B, H]
    feats[b,m,e] = hidden[b, ids[m,e]]                          [B, M, E]
    preds[b,m,o] = sum_e feats[b,m,e] * W_pred[m,o,e]           [B, M, O]
    out = preds.mean(axis=1)                                    [B, O]

The gather + per-member matmul + mean are all linear in `hidden`, so they
collapse into a single [H, O] matrix:
    A[h,o] = (1/M) * sum_{(m,e): ids[m,e]==h} W_pred[m,o,e]
    out    = tanh(x @ W_in.T) @ A

A is built on the host from the tiny W_pred/ids tensors (0.7 MB); the device
does the two matmuls + tanh. Sharding: data-parallel over batch — each of the
8 cores takes 512 rows of x; W_in^T and A are replicated. No collectives.

Device layout (per core)
------------------------
  xt  [512, 512]  bf16   x^T slice          (i on DRAM rows for K-major loads)
  wt  [512, 4096] bf16   W_in^T, replicated
  aw  [128, 320]  bf16   A packed as [p, 32*10]: aw[p, t*10+o] = A[t*128+p, o]
  out [10, 512]   f32    out^T slice (host transposes back)

  H^T tile [h=128, b=512] = (wt chunk).T @ (xt chunk), accum over 4 i-chunks
  tanh on ACT engine PSUM->SBUF (bf16)
  out^T [10, 512] = sum over 32 h-tiles of (A chunk).T @ H^T tile, PSUM accum
"""

import os

import numpy as np
import ml_dtypes

BATCH, IN_DIM, HIDDEN, N_MEMBERS, ENS, OUT = 4096, 512, 4096, 256, 64, 10
NCORES = 8
B_LOC = BATCH // NCORES      # 512 batch rows per core
HT = 128                     # h-tile height (PSUM partition dim)
NHT = HIDDEN // HT           # 32 h-tiles
NIC = IN_DIM // 128          # 4 contraction chunks for the first matmul
HG = 512                     # h-group: DMA granularity for wt (4 h-tiles)
NHG = HIDDEN // HG           # 8 groups
TPG = HG // HT               # 4 h-tiles per group

_compiled = None
LAST_RESULT = None           # BassKernelResults of the most recent run


def _build():
    from concourse import bacc, mybir
    import concourse.tile as tile

    bf16 = mybir.dt.bfloat16
    f32 = mybir.dt.float32

    nc = bacc.Bacc(
        "TRN2",
        target_bir_lowering=False,
        debug=False,
        enable_asserts=False,
        num_devices=NCORES,
    )
    xt = nc.dram_tensor("xt", [IN_DIM, B_LOC], bf16, kind="ExternalInput")
    wt = nc.dram_tensor("wt", [IN_DIM, HIDDEN], bf16, kind="ExternalInput")
    aw = nc.dram_tensor("aw", [128, NHT * OUT], bf16, kind="ExternalInput")
    out = nc.dram_tensor("out", [OUT, B_LOC], f32, kind="ExternalOutput")

    with tile.TileContext(nc) as tc:
        with (
            tc.tile_pool(name="single", bufs=1) as single,
            tc.tile_pool(name="wpool", bufs=NHG) as wpool,
            tc.tile_pool(name="hpool", bufs=NHT) as hpool,
            tc.tile_pool(name="ps", bufs=4, space="PSUM") as pspool,
            tc.tile_pool(name="psout", bufs=1, space="PSUM") as psoutp,
        ):
            # i (contraction dim) goes on partitions: view DRAM [i, ...] as
            # [p, n, ...] with i = n*128 + p.
            xt_sb = single.tile([128, NIC, B_LOC], bf16)
            nc.sync.dma_start(
                out=xt_sb[:], in_=xt.ap().rearrange("(n p) b -> p n b", p=128)
            )
            a_sb = single.tile([128, NHT * OUT], bf16)
            nc.sync.dma_start(out=a_sb[:], in_=aw.ap())

            wt_view = wt.ap().rearrange("(n p) (g h) -> p n g h", p=128, h=HG)
            wt_tiles = []
            for g in range(NHG):
                wt_g = wpool.tile([128, NIC, HG], bf16)
                nc.sync.dma_start(out=wt_g[:], in_=wt_view[:, :, g, :])
                wt_tiles.append(wt_g)

            # hidden^T tiles: H^T[t*128+p, b] = tanh(sum_i W[h,i] x[b,i])
            ht_tiles = []
            for t in range(NHT):
                g, tin = divmod(t, TPG)
                ps = pspool.tile([128, B_LOC], f32)
                for n in range(NIC):
                    nc.tensor.matmul(
                        out=ps[:],
                        lhsT=wt_tiles[g][:, n, tin * HT : (tin + 1) * HT],
                        rhs=xt_sb[:, n, :],
                        start=(n == 0),
                        stop=(n == NIC - 1),
                    )
                ht = hpool.tile([128, B_LOC], bf16)
                nc.scalar.activation(
                    out=ht[:], in_=ps[:], func=mybir.ActivationFunctionType.Tanh
                )
                ht_tiles.append(ht)

            # out^T[o, b] = sum_t (A chunk t).T @ H^T tile t, PSUM-accumulated
            ps_out = psoutp.tile([128, B_LOC], f32)
            for t in range(NHT):
                nc.tensor.matmul(
                    out=ps_out[:OUT, :],
                    lhsT=a_sb[:, t * OUT : (t + 1) * OUT],
                    rhs=ht_tiles[t][:],
                    start=(t == 0),
                    stop=(t == NHT - 1),
                )
            out_sb = single.tile([OUT, B_LOC], f32)
            nc.vector.tensor_copy(out=out_sb[:], in_=ps_out[:OUT, :])
            nc.sync.dma_start(out=out.ap(), in_=out_sb[:])

    nc.compile()
    return nc


def kernel(**inputs) -> np.ndarray:
    x = np.asarray(inputs["x"], dtype=np.float32)              # [4096, 512]
    W_in = np.asarray(inputs["W_in"], dtype=np.float32)        # [4096, 512]
    W_pred = np.asarray(inputs["W_pred"], dtype=np.float32)    # [256, 10, 64]
    ids = np.asarray(inputs["ensemble_input_ids"])             # [256, 64] int32

    # Collapse gather + einsum + mean into A[h, o].
    A = np.zeros((HIDDEN, OUT), dtype=np.float64)
    np.add.at(
        A,
        ids.reshape(-1),
        W_pred.transpose(0, 2, 1).reshape(-1, OUT).astype(np.float64),
    )
    A /= N_MEMBERS
    a_packed = np.ascontiguousarray(
        A.reshape(NHT, 128, OUT).transpose(1, 0, 2).reshape(128, NHT * OUT)
    ).astype(ml_dtypes.bfloat16)

    xt_full = np.ascontiguousarray(x.T).astype(ml_dtypes.bfloat16)   # [512, 4096]
    wt_full = np.ascontiguousarray(W_in.T).astype(ml_dtypes.bfloat16)

    global _compiled
    if _compiled is None:
        _compiled = _build()
    nc = _compiled

    in_maps = [
        {
            "xt": np.ascontiguousarray(xt_full[:, c * B_LOC : (c + 1) * B_LOC]),
            "wt": wt_full,
            "aw": a_packed,
        }
        for c in range(NCORES)
    ]

    from concourse.bass_utils import run_bass_kernel_spmd

    trace = bool(int(os.environ.get("KERNEL_TRACE", "0")))
    res = run_bass_kernel_spmd(
        nc, in_maps, core_ids=list(range(NCORES)), trace=trace
    )
    global LAST_RESULT
    LAST_RESULT = res

    out = np.empty((BATCH, OUT), dtype=np.float32)
    for c in range(NCORES):
        out[c * B_LOC : (c + 1) * B_LOC, :] = res.results[c]["out"].T
    return out


# revision 4
# speedup vs baseline: 1.0633x; 1.0633x over previous
"""EnsembleMLP fused kernel for Trainium2, 8 NeuronCores (SPMD, batch-parallel).

Math transformation
-------------------
reference:
    hidden = tanh(x @ W_in.T)                                   [B, H]
    feats[b,m,e] = hidden[b, ids[m,e]]                          [B, M, E]
    preds[b,m,o] = sum_e feats[b,m,e] * W_pred[m,o,e]           [B, M, O]
    out = preds.mean(axis=1)                                    [B, O]

The gather + per-member matmul + mean are all linear in `hidden`, so they
collapse into a single [H, O] matrix:
    A[h,o] = (1/M) * sum_{(m,e): ids[m,e]==h} W_pred[m,o,e]
    out    = tanh(x @ W_in.T) @ A

A is built on the host from the tiny W_pred/ids tensors (0.7 MB); the device
does the two matmuls + tanh. Sharding: data-parallel over batch — each of the
8 cores takes 512 rows of x; W_in^T and A are replicated. No collectives.

Device layout (per core)
------------------------
  xt  [512, 512]  bf16   x^T slice          (i on DRAM rows for K-major loads)
  wt  [512, 4096] bf16   W_in^T, replicated
  aw  [128, 320]  bf16   A packed as [p, 32*10]: aw[p, t*10+o] = A[t*128+p, o]
  out [10, 512]   f32    out^T slice (host transposes back)

  H^T tile [h=128, b=512] = (wt chunk).T @ (xt chunk), accum over 4 i-chunks
  tanh on ACT engine PSUM->SBUF (bf16)
  out^T [10, 512] = sum over 32 h-tiles of (A chunk).T @ H^T tile, PSUM accum
"""

import os

import numpy as np
import ml_dtypes

BATCH, IN_DIM, HIDDEN, N_MEMBERS, ENS, OUT = 4096, 512, 4096, 256, 64, 10
NCORES = 8
B_LOC = BATCH // NCORES      # 512 batch rows per core
HT = 128                     # h-tile height (PSUM partition dim)
NHT = HIDDEN // HT           # 32 h-tiles
NIC = IN_DIM // 128          # 4 contraction chunks for the first matmul
HG = 512                     # h-group: DMA granularity for wt (4 h-tiles)
NHG = HIDDEN // HG           # 8 groups
TPG = HG // HT               # 4 h-tiles per group

_compiled = None
LAST_RESULT = None           # BassKernelResults of the most recent run


def _build():
    from concourse import bacc, mybir
    import concourse.tile as tile

    bf16 = mybir.dt.bfloat16
    f32 = mybir.dt.float32

    nc = bacc.Bacc(
        "TRN2",
        target_bir_lowering=False,
        debug=False,
        enable_asserts=False,
        num_devices=NCORES,
    )
    xt = nc.dram_tensor("xt", [IN_DIM, B_LOC], bf16, kind="ExternalInput")
    wt = nc.dram_tensor("wt", [IN_DIM, HIDDEN], bf16, kind="ExternalInput")
    aw = nc.dram_tensor("aw", [128, NHT * OUT], bf16, kind="ExternalInput")
    out = nc.dram_tensor("out", [OUT, B_LOC], f32, kind="ExternalOutput")

    with tile.TileContext(nc) as tc:
        with (
            tc.tile_pool(name="single", bufs=1) as single,
            tc.tile_pool(name="wpool", bufs=NHT) as wpool,
            tc.tile_pool(name="hpool", bufs=NHT) as hpool,
            tc.tile_pool(name="ps", bufs=4, space="PSUM") as pspool,
            tc.tile_pool(name="psout", bufs=1, space="PSUM") as psoutp,
        ):
            # i (contraction dim) goes on partitions: view DRAM [i, ...] as
            # [p, n, ...] with i = n*128 + p.
            xt_sb = single.tile([128, NIC, B_LOC], bf16)
            nc.sync.dma_start(
                out=xt_sb[:], in_=xt.ap().rearrange("(n p) b -> p n b", p=128)
            )

            # wt is loaded in per-h-tile chunks (128 KB) so the first matmul
            # can start as soon as chunk 0 lands instead of after the whole
            # 4 MB replicated weight load.
            wt_view = wt.ap().rearrange("(n p) (t h) -> p n t h", p=128, h=HT)
            wt_tiles = []
            for t in range(NHT):
                wt_t = wpool.tile([128, NIC, HT], bf16)
                nc.sync.dma_start(out=wt_t[:], in_=wt_view[:, :, t, :])
                wt_tiles.append(wt_t)

            # aw is only needed by the trailing ensemble matmul: issue last.
            a_sb = single.tile([128, NHT * OUT], bf16)
            nc.sync.dma_start(out=a_sb[:], in_=aw.ap())

            # hidden^T tiles: H^T[t*128+p, b] = tanh(sum_i W[h,i] x[b,i])
            ht_tiles = []
            for t in range(NHT):
                ps = pspool.tile([128, B_LOC], f32)
                for n in range(NIC):
                    nc.tensor.matmul(
                        out=ps[:],
                        lhsT=wt_tiles[t][:, n, :],
                        rhs=xt_sb[:, n, :],
                        start=(n == 0),
                        stop=(n == NIC - 1),
                    )
                ht = hpool.tile([128, B_LOC], bf16)
                nc.scalar.activation(
                    out=ht[:], in_=ps[:], func=mybir.ActivationFunctionType.Tanh
                )
                ht_tiles.append(ht)

            # out^T[o, b] = sum_t (A chunk t).T @ H^T tile t. M=10 uses only
            # 10/128 PE columns, so run 4 h-tiles concurrently in distinct
            # 32-wide column groups (tile_position), each accumulating into
            # its own PSUM bank at the matching partition offset.
            ps_outs = [
                psoutp.tile([128, B_LOC], f32, name=f"ps_out{j}") for j in range(4)
            ]
            for t in range(NHT):
                j = t % 4
                nc.tensor.matmul(
                    out=ps_outs[j][32 * j : 32 * j + OUT, :],
                    lhsT=a_sb[:, t * OUT : (t + 1) * OUT],
                    rhs=ht_tiles[t][:],
                    start=(t < 4),
                    stop=(t >= NHT - 4),
                    tile_position=(0, 32 * j),
                )
            out_sb = single.tile([OUT, B_LOC], f32)
            nc.vector.tensor_copy(out=out_sb[:], in_=ps_outs[0][0:OUT, :])
            for j in range(1, 4):
                nc.vector.tensor_add(
                    out=out_sb[:],
                    in0=out_sb[:],
                    in1=ps_outs[j][32 * j : 32 * j + OUT, :],
                )
            nc.sync.dma_start(out=out.ap(), in_=out_sb[:])

    nc.compile()
    return nc


def kernel(**inputs) -> np.ndarray:
    x = np.asarray(inputs["x"], dtype=np.float32)              # [4096, 512]
    W_in = np.asarray(inputs["W_in"], dtype=np.float32)        # [4096, 512]
    W_pred = np.asarray(inputs["W_pred"], dtype=np.float32)    # [256, 10, 64]
    ids = np.asarray(inputs["ensemble_input_ids"])             # [256, 64] int32

    # Collapse gather + einsum + mean into A[h, o].
    A = np.zeros((HIDDEN, OUT), dtype=np.float64)
    np.add.at(
        A,
        ids.reshape(-1),
        W_pred.transpose(0, 2, 1).reshape(-1, OUT).astype(np.float64),
    )
    A /= N_MEMBERS
    a_packed = np.ascontiguousarray(
        A.reshape(NHT, 128, OUT).transpose(1, 0, 2).reshape(128, NHT * OUT)
    ).astype(ml_dtypes.bfloat16)

    xt_full = np.ascontiguousarray(x.T).astype(ml_dtypes.bfloat16)   # [512, 4096]
    wt_full = np.ascontiguousarray(W_in.T).astype(ml_dtypes.bfloat16)

    global _compiled
    if _compiled is None:
        _compiled = _build()
    nc = _compiled

    in_maps = [
        {
            "xt": np.ascontiguousarray(xt_full[:, c * B_LOC : (c + 1) * B_LOC]),
            "wt": wt_full,
            "aw": a_packed,
        }
        for c in range(NCORES)
    ]

    from concourse.bass_utils import run_bass_kernel_spmd

    trace = bool(int(os.environ.get("KERNEL_TRACE", "0")))
    res = run_bass_kernel_spmd(
        nc, in_maps, core_ids=list(range(NCORES)), trace=trace
    )
    global LAST_RESULT
    LAST_RESULT = res

    out = np.empty((BATCH, OUT), dtype=np.float32)
    for c in range(NCORES):
        out[c * B_LOC : (c + 1) * B_LOC, :] = res.results[c]["out"].T
    return out


# revision 8
# speedup vs baseline: 1.0969x; 1.0316x over previous
"""EnsembleMLP fused kernel for Trainium2, 8 NeuronCores (SPMD, batch-parallel).

Math transformation
-------------------
reference:
    hidden = tanh(x @ W_in.T)                                   [B, H]
    feats[b,m,e] = hidden[b, ids[m,e]]                          [B, M, E]
    preds[b,m,o] = sum_e feats[b,m,e] * W_pred[m,o,e]           [B, M, O]
    out = preds.mean(axis=1)                                    [B, O]

The gather + per-member matmul + mean are all linear in `hidden`, so they
collapse into a single [H, O] matrix:
    A[h,o] = (1/M) * sum_{(m,e): ids[m,e]==h} W_pred[m,o,e]
    out    = tanh(x @ W_in.T) @ A

A is built on the host from the tiny W_pred/ids tensors (0.7 MB); the device
does the two matmuls + tanh. Sharding: data-parallel over batch — each of the
8 cores takes 512 rows of x; W_in^T and A are replicated. No collectives.

Device layout (per core)
------------------------
All DRAM inputs are host-packed partition-major ([128, free]) so every DMA
moves >=1KB-contiguous per-partition segments:
  xt  [128, 4*512]   bf16  x^T slice:  xt[p, n*512+b] = x[c*512+b, n*128+p]
  wt  [128, 32*512]  bf16  W_in^T:     wt[p, t*512+n*128+h] = W_in[t*128+h, n*128+p]
  aw  [128, 32*10]   bf16  A packed:   aw[p, t*10+o] = A[t*128+p, o]
  out [10, 512]      f32   out^T slice (host transposes back)

  H^T tile [h=128, b=512] = (wt chunk).T @ (xt chunk), accum over 4 i-chunks
  tanh on ACT engine PSUM->SBUF (bf16)
  out^T [10, 512] = sum over 32 h-tiles of (A chunk).T @ H^T tile, 2-way
  column-tiled on the PE (M=10 uses only 10/128 PE columns), final add on DVE.
"""

import os

import numpy as np
import ml_dtypes

BATCH, IN_DIM, HIDDEN, N_MEMBERS, ENS, OUT = 4096, 512, 4096, 256, 64, 10
NCORES = 8
B_LOC = BATCH // NCORES      # 512 batch rows per core
HT = 128                     # h-tile height (PSUM partition dim)
NHT = HIDDEN // HT           # 32 h-tiles
NIC = IN_DIM // 128          # 4 contraction chunks for the first matmul
N_WARM = 7                   # dummy matmuls to lift the PE HAM clock-gate

_compiled = None
LAST_RESULT = None           # BassKernelResults of the most recent run


def _build():
    from concourse import bacc, mybir
    import concourse.tile as tile

    bf16 = mybir.dt.bfloat16
    f32 = mybir.dt.float32

    nc = bacc.Bacc(
        "TRN2",
        target_bir_lowering=False,
        debug=False,
        enable_asserts=False,
        num_devices=NCORES,
    )
    xt = nc.dram_tensor("xt", [128, NIC * B_LOC], bf16, kind="ExternalInput")
    wt = nc.dram_tensor("wt", [128, NHT * NIC * HT], bf16, kind="ExternalInput")
    aw = nc.dram_tensor("aw", [128, NHT * OUT], bf16, kind="ExternalInput")
    out = nc.dram_tensor("out", [OUT, B_LOC], f32, kind="ExternalOutput")

    with tile.TileContext(nc) as tc:
        with (
            tc.tile_pool(name="single", bufs=1) as single,
            tc.tile_pool(name="wpool", bufs=NHT) as wpool,
            tc.tile_pool(name="hpool", bufs=NHT) as hpool,
            tc.tile_pool(name="ps", bufs=4, space="PSUM") as pspool,
            tc.tile_pool(name="psout", bufs=1, space="PSUM") as psoutp,
            tc.tile_pool(name="pswarm", bufs=1, space="PSUM") as pswarm,
        ):
            # PE warm-up: the HAM clock gate holds the PE at 1.2 GHz until it
            # has been busy ~3.4us. Burn that window on zeros while the input
            # DMAs are still in flight so the real matmuls all run at 2.4 GHz.
            pass

            # x^T chunks
            xt_sb = single.tile([128, NIC, B_LOC], bf16)
            nc.sync.dma_start(
                out=xt_sb[:], in_=xt.ap().rearrange("p (n b) -> p n b", n=NIC)
            )

            # wt in per-h-tile chunks (128 KB) so the first matmul can start
            # as soon as chunk 0 lands instead of after the whole 4 MB load.
            wt_view = wt.ap().rearrange("p (t n h) -> p t n h", t=NHT, n=NIC)
            wt_tiles = []
            for t in range(NHT):
                wt_t = wpool.tile([128, NIC, HT], bf16)
                nc.sync.dma_start(out=wt_t[:], in_=wt_view[:, t, :, :])
                wt_tiles.append(wt_t)

            # aw is only needed by the trailing ensemble matmul: issue last.
            a_sb = single.tile([128, NHT * OUT], bf16)
            nc.sync.dma_start(out=a_sb[:], in_=aw.ap())

            # hidden^T tiles: H^T[t*128+p, b] = tanh(sum_i W[h,i] x[b,i])
            ht_tiles = []
            for t in range(NHT):
                ps = pspool.tile([128, B_LOC], f32)
                for n in range(NIC):
                    nc.tensor.matmul(
                        out=ps[:],
                        lhsT=wt_tiles[t][:, n, :],
                        rhs=xt_sb[:, n, :],
                        start=(n == 0),
                        stop=(n == NIC - 1),
                    )
                ht = hpool.tile([128, B_LOC], bf16)
                nc.scalar.activation(
                    out=ht[:], in_=ps[:], func=mybir.ActivationFunctionType.Tanh
                )
                ht_tiles.append(ht)

            # out^T[o, b] = sum_t (A chunk t).T @ H^T tile t. M=10 uses only
            # 10/128 PE columns, so run 2 h-tiles concurrently in distinct
            # column groups (tile_position 0 / 64), each accumulating into
            # its own PSUM bank at the matching partition offset.
            ps_outs = [
                psoutp.tile([128, B_LOC], f32, name=f"ps_out{j}") for j in range(2)
            ]
            for t in range(NHT):
                j = t % 2
                nc.tensor.matmul(
                    out=ps_outs[j][64 * j : 64 * j + OUT, :],
                    lhsT=a_sb[:, t * OUT : (t + 1) * OUT],
                    rhs=ht_tiles[t][:],
                    start=(t < 2),
                    stop=(t >= NHT - 2),
                    tile_position=(0, 64 * j),
                )
            out_sb = single.tile([OUT, B_LOC], f32)
            nc.vector.tensor_copy(out=out_sb[:], in_=ps_outs[0][0:OUT, :])
            nc.vector.tensor_add(
                out=out_sb[:],
                in0=out_sb[:],
                in1=ps_outs[1][64 : 64 + OUT, :],
            )
            nc.sync.dma_start(out=out.ap(), in_=out_sb[:])

    nc.compile()
    return nc


def kernel(**inputs) -> np.ndarray:
    x = np.asarray(inputs["x"], dtype=np.float32)              # [4096, 512]
    W_in = np.asarray(inputs["W_in"], dtype=np.float32)        # [4096, 512]
    W_pred = np.asarray(inputs["W_pred"], dtype=np.float32)    # [256, 10, 64]
    ids = np.asarray(inputs["ensemble_input_ids"])             # [256, 64] int32

    # Collapse gather + einsum + mean into A[h, o].
    A = np.zeros((HIDDEN, OUT), dtype=np.float64)
    np.add.at(
        A,
        ids.reshape(-1),
        W_pred.transpose(0, 2, 1).reshape(-1, OUT).astype(np.float64),
    )
    A /= N_MEMBERS
    a_packed = np.ascontiguousarray(
        A.reshape(NHT, 128, OUT).transpose(1, 0, 2).reshape(128, NHT * OUT)
    ).astype(ml_dtypes.bfloat16)

    xt_bf = x.T.astype(ml_dtypes.bfloat16)                     # [512, 4096]
    wt_bf = W_in.T.astype(ml_dtypes.bfloat16)                  # [512, 4096]
    # wt packed partition-major: [p, t*512 + n*128 + h] = W_in.T[n*128+p, t*128+h]
    wt_packed = np.ascontiguousarray(
        wt_bf.reshape(NIC, 128, NHT, HT).transpose(1, 2, 0, 3).reshape(128, -1)
    )

    global _compiled
    if _compiled is None:
        _compiled = _build()
    nc = _compiled

    in_maps = []
    for c in range(NCORES):
        xs = xt_bf[:, c * B_LOC : (c + 1) * B_LOC]             # [512, 512]
        xt_packed = np.ascontiguousarray(
            xs.reshape(NIC, 128, B_LOC).transpose(1, 0, 2).reshape(128, -1)
        )
        in_maps.append({"xt": xt_packed, "wt": wt_packed, "aw": a_packed})

    from concourse.bass_utils import run_bass_kernel_spmd

    trace = bool(int(os.environ.get("KERNEL_TRACE", "0")))
    res = run_bass_kernel_spmd(
        nc, in_maps, core_ids=list(range(NCORES)), trace=trace
    )
    global LAST_RESULT
    LAST_RESULT = res

    out = np.empty((BATCH, OUT), dtype=np.float32)
    for c in range(NCORES):
        out[c * B_LOC : (c + 1) * B_LOC, :] = res.results[c]["out"].T
    return out
